# revision 44
# baseline (speedup 1.0000x reference)
"""Trainium2 Bass kernel for nn_GammaNeuronNet (conductance-based neuron network).

Strategy (v2)
-------------
N=4096 neurons, 300 sequential timesteps. Per step, three matvecs against two
constant 4096x4096 matrices (G_syn used twice for s and s*E_syn, G_gap once
for V), then an elementwise state update of (V, s).

* Row-partition G_syn/G_gap across the 8 cores (512 rows each). Both shards
  stay SBUF-resident for the whole kernel (fp8 or bf16).
* x-stationary matmuls: lhsT = 2 columns of per-neuron state, rhs = G^T tile
  streamed, PSUM out [2,512] accumulated over 64 merged k-tiles
  (col0 = co_syn, col1 = int_syn + int_gap).  With fp8, DoubleRow perf mode
  processes two k-tiles per instruction (2x rate); the fp8 scale factor S on
  G is folded into the precomputed constants (cgl * S, dt/S) so no descale
  ops are needed.
* The s-part of the state does NOT depend on the matvecs, so every core
  redundantly computes the FULL s / s*E_syn vectors locally each step.  Only
  V needs to be exchanged: one [4,128] bf16 DMA -> 8-core AllGather ->
  [32,128] -> one gather-DMA into the per-neuron layout.
* k-tile order puts all G_syn tiles (which need only the locally computed
  s/sE) first, G_gap (which needs the gathered V) last, so the V AllGather
  of step i hides behind the G_syn half of step i+1's matmul burst.

Layouts: "xw layout" puts neuron n at [n//32 (partition), n%32].  SSE holds
[s | sE] and ZV [zeros | V] as two [128,32] halves in plain xw order; fp8
DoubleRow pairs k-tiles (j, j+16) so the pair stride is 16 columns,
satisfying the LDWEIGHTS ISA rule (pair step % 16 == 0) with no permutation.
The V update runs directly in the matmul output layout ([2,512], den/num
rows) so the exchange path has no PSUM copies and no PE transposes: the
updated own-V row [1,512] is cast and DMA'd straight out, and the gathered
[128,32] buffer is the V half of ZV verbatim.
"""

import os
import numpy as np
import ml_dtypes

N = 4096
NCORES = 8
ROWS = N // NCORES            # 512 matrix rows per core
MT = ROWS // 128              # 4 m-tiles of 128 rows
KTM = N // 128                # 32 k-tiles per matrix
KT = 2 * KTM                  # 64 merged k-tiles (G_syn then G_gap)
BETA, V_TH, A_R, A_D = 0.125, -15.0, 1.0, 5.0

USE_FP8 = os.environ.get("GAMMA_FP8", "1") == "1"
FP8_SCALE = 2.0 ** 17         # G values <= 1e-3 -> scaled <= ~131 (fp8e4 max 240)
# dummy matmuls issued between the G_syn and G_gap halves: they fill the PE
# idle window while the V AllGather completes, keeping the array at full
# clock (first ~14 matmuls after an idle run at half rate otherwise)
N_WARM = int(os.environ.get("GAMMA_WARM", "14"))

_cache = {}
last_results = None


def _n_steps(timestep, runtime):
    # replicate the reference's float-accumulation loop exactly
    t, n = 0.0, 0
    while t < runtime:
        t += timestep
        n += 1
    return n


def _build(n_steps: int, dt: float, use_fp8: bool):
    import concourse.bacc as bacc
    import concourse.mybir as mybir
    import concourse.tile as tile

    f32 = mybir.dt.float32
    bf16 = mybir.dt.bfloat16
    xdt = mybir.dt.float8e4 if use_fp8 else bf16
    S = FP8_SCALE if use_fp8 else 1.0

    nc = bacc.Bacc("TRN2", target_bir_lowering=False, debug=False,
                   num_devices=NCORES)

    w_d = nc.dram_tensor("w_in", [128, KT * ROWS], xdt, kind="ExternalInput")
    sse0_d = nc.dram_tensor("sse0_in", [128, 64], xdt, kind="ExternalInput")
    zv0_d = nc.dram_tensor("zv0_in", [128, 64], xdt, kind="ExternalInput")
    sf0_d = nc.dram_tensor("sf0_in", [128, 32], f32, kind="ExternalInput")
    vs0_d = nc.dram_tensor("vs0_in", [1, ROWS], f32, kind="ExternalInput")
    cgl_d = nc.dram_tensor("cgl_in", [2, ROWS], f32, kind="ExternalInput")
    esyn_d = nc.dram_tensor("esyn_in", [128, 32], f32, kind="ExternalInput")
    vout_d = nc.dram_tensor("v_out", [1, ROWS], f32, kind="ExternalOutput")

    rg = [list(range(NCORES))]
    Sigmoid = mybir.ActivationFunctionType.Sigmoid
    Copy = mybir.ActivationFunctionType.Copy
    DR = mybir.MatmulPerfMode.DoubleRow
    Alu = mybir.AluOpType

    ar_dt = float(A_R) * dt              # u = ar_dt * sigmoid(...)
    c1 = 1.0 - float(A_D) * dt           # s_new = s*(c1 - u) + u
    sig_scale = float(BETA)
    sig_bias = -float(BETA) * float(V_TH)
    dtS = dt / S                         # vstep = dv_s * min(dt/S, 1/den_s)

    with tile.TileContext(nc) as tc:
        with (
            tc.tile_pool(name="const", bufs=1) as constp,
            tc.tile_pool(name="wpool", bufs=1) as wp,
            tc.tile_pool(name="state", bufs=1) as stp,
            tc.tile_pool(name="ew", bufs=2) as ewp,
            tc.tile_pool(name="mm", bufs=1, space="PSUM") as mmp,
            tc.tile_pool(name="dram", bufs=2, space="DRAM") as dramp,
        ):
            w_sb = wp.tile([128, KT * ROWS], xdt)
            nc.sync.dma_start(w_sb[:], w_d[:])
            cgl_sb = constp.tile([2, ROWS], f32)
            nc.sync.dma_start(cgl_sb[:], cgl_d[:])
            esyn_sb = constp.tile([128, 32], f32)
            nc.sync.dma_start(esyn_sb[:], esyn_d[:])
            sigb_sb = constp.tile([128, 1], f32)
            nc.vector.memset(sigb_sb[:], sig_bias)

            # persistent double-buffered state tiles (index = step parity)
            SSE = [stp.tile([128, 64], xdt, name=f"sse{j}") for j in range(2)]
            ZV = [stp.tile([128, 64], xdt, name=f"zv{j}") for j in range(2)]
            sf = [stp.tile([128, 32], f32, name=f"sf{j}") for j in range(2)]
            vs = [stp.tile([1, ROWS], f32, name=f"vs{j}") for j in range(2)]
            nc.sync.dma_start(SSE[0][:], sse0_d[:])
            nc.sync.dma_start(ZV[0][:], zv0_d[:])
            nc.sync.dma_start(ZV[1][:], zv0_d[:])   # for the zero columns
            nc.sync.dma_start(sf[0][:], sf0_d[:])
            nc.sync.dma_start(vs[0][:], vs0_d[:])

            ccin = [dramp.tile([1, ROWS], xdt, tag="ccin", name=f"ccin{j}")
                    for j in range(2)]

            # mix rows [-V; 1]: multiplying [den; num] by it and partition-
            # reducing gives num - V*den without any unaligned-partition AP
            mix = stp.tile([2, ROWS], f32, name="mix")
            nc.vector.memset(mix[:], 1.0)
            mixv = mix[0:1, :]
            nc.vector.tensor_scalar_mul(mixv, vs[0][:], -1.0)

            mm_ps = [mmp.tile([2, ROWS], f32, name=f"mm{j}") for j in range(2)]
            warm_ps = mmp.tile([2, ROWS], f32, name="warm")

            for i in range(n_steps):
                q, nq = i % 2, (i + 1) % 2
                last = i == n_steps - 1
                mm = mm_ps[q]

                # ---- matmul burst: G_syn k-tiles first (local s/sE), then
                #      warm-up dummies, then G_gap (needs the gathered V)
                def half(lhs_tile, first, start, stop, out, n_mm=None):
                    if use_fp8:
                        # pair j covers k-tiles (j, j+16): lhsT cols 16 apart
                        # [p, i(pair, step 16), m(step 32)] at offset j
                        lr = lhs_tile[:].rearrange(
                            "p (m i j) -> p j i m", m=2, i=2)
                        rng = range(KTM // 2) if n_mm is None else range(n_mm)
                        for j in rng:
                            jb = first // 2 + j
                            nc.tensor.matmul(
                                out[:],
                                lr[:, j],
                                w_sb[:, jb * 1024:(jb + 1) * 1024].rearrange(
                                    "p (i n) -> p i n", i=2),
                                start=(start and j == 0),
                                stop=(stop and j == rng[-1]),
                                perf_mode=DR,
                            )
                    else:
                        lr = lhs_tile[:].rearrange("p (m c) -> p c m", m=2)
                        rng = range(KTM) if n_mm is None else range(n_mm)
                        for t in rng:
                            kt = first + t
                            nc.tensor.matmul(
                                out[:],
                                lr[:, t],
                                w_sb[:, kt * ROWS:(kt + 1) * ROWS],
                                start=(start and t == 0),
                                stop=(stop and t == rng[-1]),
                            )

                half(SSE[q], 0, True, False, mm)
                for wj in range(N_WARM):
                    half(SSE[q], 0, True, True, warm_ps, n_mm=1)
                half(ZV[q], KTM, False, True, mm)

                # ---- full-vector s-chain for step i+1 (scalar + gpsimd;
                #      overlaps the matmul burst; only needs Vg[q], sf[q])
                if not last:
                    sg = ewp.tile([128, 32], f32, tag="sg")
                    u = ewp.tile([128, 32], f32, tag="u")
                    w_ = ewp.tile([128, 32], f32, tag="w")
                    p2 = ewp.tile([128, 32], f32, tag="p2")
                    se = ewp.tile([128, 32], f32, tag="se")
                    nc.scalar.activation(sg[:], ZV[q][:, 32:64], Sigmoid,
                                         bias=sigb_sb[:, 0:1], scale=sig_scale)
                    nc.scalar.activation(u[:], sg[:], Copy, bias=0.0,
                                         scale=ar_dt)
                    nc.scalar.activation(w_[:], u[:], Copy, bias=c1,
                                         scale=-1.0)
                    nc.vector.tensor_mul(p2[:], sf[q][:], w_[:])
                    nc.vector.tensor_add(sf[nq][:], p2[:], u[:])
                    nc.vector.tensor_mul(se[:], sf[nq][:], esyn_sb[:])
                    nc.gpsimd.tensor_copy(SSE[nq][:, 0:32], sf[nq][:])
                    nc.gpsimd.tensor_copy(SSE[nq][:, 32:64], se[:])

                # Everything from here to the ZV DMA is the V-exchange
                # critical path: high priority so the Tile scheduler runs it
                # the moment it is ready, and entirely off the PE — the V
                # update runs in the matmul's own [2,512] layout (den row 0,
                # num row 1), reading PSUM directly; the result row [1,512]
                # is already in exchange order.
                with tc.high_priority():
                    # all partition bases stay at 0 (engines require
                    # 32-aligned bases): dv = num - V*den is formed by
                    # multiplying [den; num] rows with [-V; 1] and partition-
                    # reducing on gpsimd, concurrent with the vector recip.
                    dn = ewp.tile([2, ROWS], f32, tag="dn")
                    P = ewp.tile([2, ROWS], f32, tag="pp")
                    t1 = ewp.tile([1, ROWS], f32, tag="t1")
                    dv = ewp.tile([1, ROWS], f32, tag="dv")
                    r = ewp.tile([1, ROWS], f32, tag="r")
                    nc.vector.tensor_add(dn[:], mm[:], cgl_sb[:])
                    den = dn[0:1, :]
                    nc.gpsimd.tensor_mul(P[:], dn[:], mix[:])
                    nc.vector.reciprocal(r[:], den)
                    nc.gpsimd.tensor_reduce(dv[:], P[:], mybir.AxisListType.C,
                                            Alu.add)
                    # vstep = (min(1/den, dt/S)) * dv
                    nc.vector.scalar_tensor_tensor(
                        t1[:], r[:], dtS, dv[:], op0=Alu.min, op1=Alu.mult)
                    nc.vector.tensor_add(vs[nq][:], vs[q][:], t1[:])

                    if last:
                        nc.sync.dma_start(vout_d[:], vs[nq][:])
                        break

                    # ---- V exchange: cast own row to matmul dtype, DMA out,
                    #      AllGather, DMA the gathered [128,32] into ZV's
                    #      V half (identical layouts end to end).
                    ccx = ewp.tile([1, ROWS], xdt, tag="ccx")
                    nc.vector.tensor_copy(ccx[:], vs[nq][:])
                    nc.sync.dma_start(ccin[nq][:], ccx[:])
                    # -V for the next step's tail, off the critical path
                    nc.vector.tensor_scalar_mul(mixv, vs[nq][:], -1.0)

                    ccout = nc.dram_tensor(f"ccout{i}", [128, 32], xdt,
                                           addr_space="Shared")
                    nc.gpsimd.collective_compute(
                        "AllGather",
                        mybir.AluOpType.bypass,
                        replica_groups=rg,
                        ins=[ccin[nq][:].opt()],
                        outs=[ccout[:].opt()],
                    )
                    nc.sync.dma_start(ZV[nq][:, 32:64], ccout[:])

    nc.compile()
    return nc


def _prep(input_V, G_leak, E_leak, G_syn, E_syn, G_gap, use_fp8):
    iv = np.asarray(input_V, np.float32).reshape(-1)
    G_leak = np.asarray(G_leak, np.float32)
    E_leak = np.asarray(E_leak, np.float32)
    G_syn = np.asarray(G_syn, np.float32)
    E_syn = np.asarray(E_syn, np.float32)
    G_gap = np.asarray(G_gap, np.float32)
    in_len = iv.shape[0]
    S = np.float32(FP8_SCALE if use_fp8 else 1.0)
    wt = ml_dtypes.float8_e4m3fn if use_fp8 else ml_dtypes.bfloat16

    in_avg = np.float32(iv.mean(dtype=np.float32))
    V0 = np.concatenate([iv, np.full(N - in_len, in_avg, np.float32)])
    x = (BETA * (V0 - V_TH)).astype(np.float32)
    sig = (1.0 / (1.0 + np.exp(-x, dtype=np.float32))).astype(np.float32)
    s0 = (A_R * sig / (A_R * sig + A_D)).astype(np.float32)
    sE0 = (s0 * E_syn).astype(np.float32)
    co_gap = G_gap.sum(axis=1, dtype=np.float32)
    c0_full = ((G_leak + co_gap) * S).astype(np.float32)
    gle_full = (G_leak * E_leak * S).astype(np.float32)

    def xwl(v):
        # [N] full vector -> [128, 32] xw layout (neuron n at [n//32, n%32])
        return np.ascontiguousarray(v.reshape(128, 32))

    sse0 = np.ascontiguousarray(
        np.concatenate([xwl(s0), xwl(sE0)], axis=1).astype(wt))
    zv0 = np.ascontiguousarray(
        np.concatenate([np.zeros((128, 32), np.float32), xwl(V0)],
                       axis=1).astype(wt))
    sf0 = xwl(s0)
    esyn_full = xwl(E_syn)

    Gs = (G_syn * S).astype(wt)
    Gg = (G_gap * S).astype(wt)

    in_maps = []
    for c in range(NCORES):
        rows = slice(c * ROWS, (c + 1) * ROWS)
        A_s = Gs[rows, :].reshape(ROWS, 128, 32)   # [n, p, t], k = 32p + t
        A_g = Gg[rows, :].reshape(ROWS, 128, 32)
        Ws = np.transpose(A_s, (1, 2, 0))          # [p, t, n]
        Wg = np.transpose(A_g, (1, 2, 0))
        if use_fp8:
            # DoubleRow pair block j = k-tiles (j, j+16) back to back
            Ws = Ws.reshape(128, 2, 16, ROWS).transpose(0, 2, 1, 3)
            Wg = Wg.reshape(128, 2, 16, ROWS).transpose(0, 2, 1, 3)
        W = np.ascontiguousarray(
            np.concatenate([Ws.reshape(128, KTM, ROWS),
                            Wg.reshape(128, KTM, ROWS)], axis=1)
        ).reshape(128, KT * ROWS)
        cgl = np.stack([c0_full[rows], gle_full[rows]])
        in_maps.append({
            "w_in": W,
            "sse0_in": sse0,
            "zv0_in": zv0,
            "sf0_in": sf0,
            "vs0_in": np.ascontiguousarray(V0[rows].reshape(1, ROWS)),
            "cgl_in": np.ascontiguousarray(cgl),
            "esyn_in": esyn_full,
        })
    return in_maps, in_len


def kernel(input_V, G_leak, E_leak, G_syn, E_syn, G_gap, timestep, runtime):
    global last_results
    from concourse.bass_utils import run_bass_kernel_spmd

    dt = float(np.asarray(timestep))
    rt = float(np.asarray(runtime))
    n_steps = _n_steps(dt, rt)

    key = (n_steps, dt, USE_FP8, N_WARM)
    if key not in _cache:
        _cache[key] = _build(n_steps, dt, USE_FP8)
    nc = _cache[key]

    in_maps, in_len = _prep(input_V, G_leak, E_leak, G_syn, E_syn, G_gap,
                            USE_FP8)
    trace = os.environ.get("GAMMA_TRACE", "0") == "1"
    res = run_bass_kernel_spmd(
        nc, in_maps, core_ids=list(range(NCORES)), trace=trace
    )
    last_results = res

    V = np.concatenate(
        [np.asarray(res.results[c]["v_out"]).reshape(ROWS)
         for c in range(NCORES)]
    ).astype(np.float32)
    V[in_len:] = 0.0
    return V
